# revision 1
# baseline (speedup 1.0000x reference)
"""DCGRU cell on 8 Trainium2 NeuronCores — data-parallel over batch.

Problem: nn_DCGRUCell (B=64, N=1024, D_IN=2, U=64, K=2, 2 supports).
Sharding: batch 64 -> 8 cores x 8 local batches (j). Supports + weights
replicated per core; everything else fully local, no collectives.

v2: all layout transposes routed through DRAM-staged DMA xbar transposes
(bf16) instead of PE transpose instructions; xi/A family computed in the
transposed domain via dual matmuls against S^T.

Per-core layout:
  hall[nt] [128, 3072] bf16  SBUF row-domain; cols c = j*384 + slot*64 + u
           slots gconv1: 0=x0h(hx) 1=x1 2=x2 3=x3 4=x4 5=zero-pad
           slots gconv2: 1=x0'h(r*hx) 2=x1' 3=x2' 4=x3' 5=x4' (0=stale)
  hsc      [1024, 3072] bf16 DRAM mirror of hall (written after each evac);
           hT_{j,g} [128,1024] = dma-transpose of hsc 128-col chunk q=j*3+g
  aT_{jp}  [74, 1024] bf16   rows (j%2)*64+(mm*2+f): transposed A family
  value_j  [128, 1024] bf16  gconv1 rows 0:64=r, 64:128=u; gconv2: 0:64=c
Projection per (j, n-chunk): 3 H-matmuls (k=128, zero-padded weights for
pad/stale slots) + 1 A-matmul (k=10). Final blend in row domain after
DMA-transposing r/u/c back.
"""

import numpy as np
import ml_dtypes

import concourse.bass as bass
import concourse.tile as tile
import concourse.mybir as mybir
from concourse import bacc
from concourse.bass_utils import run_bass_kernel_spmd

BF = mybir.dt.bfloat16
F32 = mybir.dt.float32
AF = mybir.ActivationFunctionType
OP = mybir.AluOpType

B, N, D_IN, U = 64, 1024, 2, 64
NCORES, J = 8, 8
NT = 8
NSLOT = 6
HCOLS = J * NSLOT * U   # 3072
O1, O2 = 2 * U, U

_CACHE = {}
STAGE = 99  # build cutoff for profiling components


def _build(reps=1):
    nc = bacc.Bacc(None)

    s0t_d = nc.dram_tensor("s0t", [N, N], BF, kind="ExternalInput")
    s1t_d = nc.dram_tensor("s1t", [N, N], BF, kind="ExternalInput")
    hxr_d = nc.dram_tensor("hxr", [N, J * U], BF, kind="ExternalInput")
    hxrf_d = nc.dram_tensor("hxrf", [N, J * U], F32, kind="ExternalInput")
    a0r_d = nc.dram_tensor("a0r", [N, 16], BF, kind="ExternalInput")
    a0t_d = nc.dram_tensor("a0t", [16, N], BF, kind="ExternalInput")
    wo_g_d = [nc.dram_tensor(f"wo_g{g}", [128, O1], BF, kind="ExternalInput") for g in range(5)]
    wu_g_d = [nc.dram_tensor(f"wu_g{g}", [128, O2], BF, kind="ExternalInput") for g in range(5)]
    wa_o_d = nc.dram_tensor("wa_o", [128, O1], BF, kind="ExternalInput")
    wa_u_d = nc.dram_tensor("wa_u", [128, O2], BF, kind="ExternalInput")
    b_o_d = nc.dram_tensor("b_o", [O1, 1], F32, kind="ExternalInput")
    b_u_d = nc.dram_tensor("b_u", [O2, 1], F32, kind="ExternalInput")
    out_d = nc.dram_tensor("out", [J, N * U], F32, kind="ExternalOutput")

    with tile.TileContext(nc) as tc:
        with (
            tc.tile_pool(name="const", bufs=1) as cp,
            tc.tile_pool(name="hall", bufs=1) as hp,
            tc.tile_pool(name="misc", bufs=1) as mp,
            tc.tile_pool(name="ht", bufs=12) as htp,
            tc.tile_pool(name="stream", bufs=2) as sp,
            tc.tile_pool(name="dram", bufs=1, space="DRAM") as dp,
            tc.tile_pool(name="pd", bufs=4, space="PSUM") as pdp,
            tc.tile_pool(name="pa", bufs=2, space="PSUM") as pap,
            tc.tile_pool(name="pp", bufs=2, space="PSUM") as ppp,
        ):
            env = {}
            # ---- constants ----
            s0t = [cp.tile([128, N], BF, name=f"s0t{k}") for k in range(NT)]
            s1t = [cp.tile([128, N], BF, name=f"s1t{k}") for k in range(NT)]
            for k in range(NT):
                nc.sync.dma_start(s0t[k], s0t_d[k * 128:(k + 1) * 128, :])
                nc.sync.dma_start(s1t[k], s1t_d[k * 128:(k + 1) * 128, :])
            wo_g = [cp.tile([128, O1], BF, name=f"wo_g{g}") for g in range(5)]
            wu_g = [cp.tile([128, O2], BF, name=f"wu_g{g}") for g in range(5)]
            for g in range(5):
                nc.sync.dma_start(wo_g[g], wo_g_d[g][:, :])
                nc.sync.dma_start(wu_g[g], wu_g_d[g][:, :])
            wa_o = cp.tile([128, O1], BF, name="wa_o")
            wa_u = cp.tile([128, O2], BF, name="wa_u")
            nc.sync.dma_start(wa_o, wa_o_d[:, :])
            nc.sync.dma_start(wa_u, wa_u_d[:, :])
            b_o = cp.tile([O1, 1], F32, name="b_o")
            b_u = cp.tile([O2, 1], F32, name="b_u")
            nc.sync.dma_start(b_o, b_o_d[:, :])
            nc.sync.dma_start(b_u, b_u_d[:, :])

            # ---- persistent state ----
            hall = [hp.tile([128, HCOLS], BF, name=f"hall{t}") for t in range(NT)]
            h4 = [h.rearrange("p (s j u) -> p s j u", j=J, s=NSLOT) for h in hall]
            hxrf = [mp.tile([128, J * U], F32, name=f"hxrf{t}") for t in range(NT)]
            a0r = [mp.tile([128, 16], BF, name=f"a0r{t}") for t in range(NT)]
            a0t = mp.tile([16, N], BF, name="a0t")
            nc.sync.dma_start(a0t, a0t_d[:, :])
            for t in range(NT):
                nc.gpsimd.memset(hall[t][:, 5 * 512:6 * 512], 0.0)
                nc.sync.dma_start(hall[t][:, 0:512],
                                  hxr_d[t * 128:(t + 1) * 128, :])
                nc.sync.dma_start(hxrf[t], hxrf_d[t * 128:(t + 1) * 128, :])
                nc.sync.dma_start(a0r[t], a0r_d[t * 128:(t + 1) * 128, :])

            hsc = dp.tile([N, HCOLS], BF, name="hsc")
            hs4 = hsc.rearrange("n (s j u) -> n s j u", j=J, s=NSLOT)
            vdram = dp.tile([3, J * U, N], BF, name="vdram")  # 0=r 1=u 2=c
            for t in range(NT):
                # mirror slot0 + zero slot5 into the DRAM copy
                nc.sync.dma_start(hsc[t * 128:(t + 1) * 128, 0:512],
                                  hall[t][:, 0:512])
                nc.sync.dma_start(hsc[t * 128:(t + 1) * 128, 5 * 512:6 * 512],
                                  hall[t][:, 5 * 512:6 * 512])

            aT = [mp.tile([74, N], BF, name=f"aT{p}") for p in range(4)]
            value = [mp.tile([128, N], BF, name=f"value{j}") for j in range(J)]
            stg = [mp.tile([16, N], BF, name=f"stg{m}") for m in range(1, 5)]  # a1..a4 ^T
            a1r = [mp.tile([128, 16], BF, name=f"a1r{t}") for t in range(NT)]
            a3r = [mp.tile([128, 16], BF, name=f"a3r{t}") for t in range(NT)]
            adr = dp.tile([2, 16, N], BF, name="adr")

            env.update(locals())
            for rep in range(reps):
                _emit_body(env, rep)
    nc.compile()
    return nc


def _emit_body(env, rep):
    nc = env["nc"]
    s0t, s1t = env["s0t"], env["s1t"]
    wo_g, wu_g = env["wo_g"], env["wu_g"]
    wa_o, wa_u = env["wa_o"], env["wa_u"]
    b_o, b_u = env["b_o"], env["b_u"]
    hall, h4 = env["hall"], env["h4"]
    hs4 = env["hs4"]
    hsc = env["hsc"]
    vdram = env["vdram"]
    hxrf, aT, value = env["hxrf"], env["aT"], env["value"]
    a0r, a0t, stg, a1r, a3r, adr = (env["a0r"], env["a0t"], env["stg"],
                                    env["a1r"], env["a3r"], env["adr"])
    pdp, pap, ppp = env["pdp"], env["pap"], env["ppp"]
    htp, sp = env["htp"], env["sp"]
    out_d = env["out_d"]
    R = f"r{rep}"

    def diffuse(steps):
        for si, (src, dst, st, base) in enumerate(steps):
            for it in range(NT):
                pd = pdp.tile([128, J * U], F32, name=f"pd{R}_{si}_{it}", tag="pd")
                for kt in range(NT):
                    nc.tensor.matmul(
                        pd, st[kt][:, it * 128:(it + 1) * 128],
                        hall[kt][:, src * 512:(src + 1) * 512],
                        start=(kt == 0), stop=(kt == NT - 1),
                    )
                if base is None:
                    nc.any.tensor_copy(hall[it][:, dst * 512:(dst + 1) * 512], pd)
                else:
                    nc.vector.scalar_tensor_tensor(
                        out=hall[it][:, dst * 512:(dst + 1) * 512], in0=pd, scalar=2.0,
                        in1=hall[it][:, base * 512:(base + 1) * 512],
                        op0=OP.mult, op1=OP.subtract,
                    )
                nc.sync.dma_start(
                    hsc[it * 128:(it + 1) * 128, dst * 512:(dst + 1) * 512],
                    hall[it][:, dst * 512:(dst + 1) * 512],
                )

    def a_family():
        # a1^T, a2^T (S0) then a3^T, a4^T (S1), all [16, 1024] bf16 tiles.
        for sidx, (st, lo_t, row_t, s1_t, s2_t) in enumerate(
            ((s0t, 0, a1r, stg[0], stg[1]), (s1t, 1, a3r, stg[2], stg[3]))
        ):
            for c2 in range(2):
                ps = env["pap"].tile([16, 512], F32, name=f"pa1{R}_{sidx}_{c2}", tag="pa")
                for kt in range(NT):
                    nc.tensor.matmul(
                        ps, a0r[kt], st[kt][:, c2 * 512:(c2 + 1) * 512],
                        start=(kt == 0), stop=(kt == NT - 1),
                    )
                nc.any.tensor_copy(s1_t[:, c2 * 512:(c2 + 1) * 512], ps)
            # row-form of a1 via DRAM dma transpose
            nc.sync.dma_start(adr[lo_t], s1_t)
            for t in range(NT):
                nc.sync.dma_start(row_t[t], adr[lo_t, :, t * 128:(t + 1) * 128],
                                  transpose=True)
            for c2 in range(2):
                ps = env["pap"].tile([16, 512], F32, name=f"pa2{R}_{sidx}_{c2}", tag="pa")
                for kt in range(NT):
                    nc.tensor.matmul(
                        ps, row_t[kt], st[kt][:, c2 * 512:(c2 + 1) * 512],
                        start=(kt == 0), stop=(kt == NT - 1),
                    )
                nc.vector.scalar_tensor_tensor(
                    out=s2_t[:, c2 * 512:(c2 + 1) * 512], in0=ps, scalar=2.0,
                    in1=a0t[:, c2 * 512:(c2 + 1) * 512],
                    op0=OP.mult, op1=OP.subtract,
                )
        # pack aT pair tiles: rows (j%2)*64 + mm*2 + f
        for j in range(J):
            jo = (j % 2) * 64
            nc.sync.dma_start(aT[j // 2][jo:jo + 2, :], a0t[2 * j:2 * j + 2, :])
            for m in range(4):
                nc.sync.dma_start(aT[j // 2][jo + 2 * m + 2:jo + 2 * m + 4, :],
                                  stg[m][2 * j:2 * j + 2, :])

    def transposes(jp, gc):
        # chunk (slot, jp): cols slot*512 + jp*128 .. +128 of hsc
        slots = range(0, 5) if gc == 0 else range(1, 6)
        hts = {}
        for s_ in slots:
            q = s_ * 4 + jp
            ht = htp.tile([128, N], BF, name=f"ht{R}_{gc}_{jp}_{s_}", tag="ht")
            nc.sync.dma_start(ht, hsc[:, q * 128:(q + 1) * 128], transpose=True)
            hts[s_] = ht
        return hts

    def project(gc, j, hts):
        wg, wa, ob = (wo_g, wa_o, O1) if gc == 0 else (wu_g, wa_u, O2)
        slots = list(range(0, 5)) if gc == 0 else list(range(1, 6))
        jo = (j % 2) * 64
        for c2 in range(2):
            pp = ppp.tile([ob, 512], F32, name=f"pp{R}_{gc}_{j}_{c2}", tag="pp")
            cs = slice(c2 * 512, (c2 + 1) * 512)
            for i, s_ in enumerate(slots):
                nc.tensor.matmul(pp, wg[i][jo:jo + 64, :],
                                 hts[s_][jo:jo + 64, cs],
                                 start=(i == 0), stop=False)
            nc.tensor.matmul(pp, wa[jo:jo + 10, :], aT[j // 2][jo:jo + 10, cs],
                             start=False, stop=True)
            if gc == 0:
                nc.scalar.activation(out=value[j][:, cs], in_=pp,
                                     func=AF.Sigmoid, bias=b_o, scale=1.0)
            else:
                nc.scalar.activation(out=value[j][0:O2, cs], in_=pp,
                                     func=AF.Tanh, bias=b_u, scale=1.0)

    # ================= gconv 1 =================
    if STAGE < 1:
        return
    diffuse([(0, 1, s0t, None), (1, 2, s0t, 0),
             (0, 3, s1t, None), (3, 4, s1t, 0)])
    if STAGE < 2:
        return
    if rep == 0:
        a_family()
    if STAGE < 3:
        return

    for jp in range(4):
        hts = transposes(jp, 0)
        for j in (2 * jp, 2 * jp + 1):
            project(0, j, hts)
    if STAGE < 4:
        return

    # dump r and u to DRAM, rebuild row-form r; gconv2 slot1 = r * hx
    for j in range(J):
        nc.sync.dma_start(vdram[0, j * 64:(j + 1) * 64, :], value[j][0:64, :])
        nc.sync.dma_start(vdram[1, j * 64:(j + 1) * 64, :], value[j][64:128, :])
    for t in range(NT):
        rr = sp.tile([128, J * U], BF, name=f"rr{R}_{t}", tag="rr")
        nc.sync.dma_start(rr, vdram[0, :, t * 128:(t + 1) * 128], transpose=True)
        nc.vector.tensor_mul(hall[t][:, 512:1024], rr, hall[t][:, 0:512])
        nc.sync.dma_start(hsc[t * 128:(t + 1) * 128, 512:1024],
                          hall[t][:, 512:1024])

    if STAGE < 5:
        return
    # ================= gconv 2 =================
    diffuse([(1, 2, s0t, None), (2, 3, s0t, 1),
             (1, 4, s1t, None), (4, 5, s1t, 1)])
    if STAGE < 6:
        return

    for jp in range(4):
        hts = transposes(jp, 1)
        for j in (2 * jp, 2 * jp + 1):
            project(1, j, hts)
    if STAGE < 7:
        return

    for j in range(J):
        nc.sync.dma_start(vdram[2, j * 64:(j + 1) * 64, :], value[j][0:64, :])
    if STAGE < 8:
        return

    # ---- final blend in row domain: out = c + u*(hx - c) ----
    for t in range(NT):
        ur = sp.tile([128, J * U], BF, name=f"ur{R}_{t}", tag="ur")
        cr = sp.tile([128, J * U], BF, name=f"cr{R}_{t}", tag="cr")
        nc.sync.dma_start(ur, vdram[1, :, t * 128:(t + 1) * 128], transpose=True)
        nc.sync.dma_start(cr, vdram[2, :, t * 128:(t + 1) * 128], transpose=True)
        tmp = sp.tile([128, J * U], F32, name=f"tmp{R}_{t}", tag="tmp")
        nc.vector.tensor_sub(tmp, hxrf[t], cr)
        tmp2 = sp.tile([128, J * U], F32, name=f"tmp2{R}_{t}", tag="tmp2")
        nc.vector.tensor_mul(tmp2, ur, tmp)
        orow = sp.tile([128, J * U], F32, name=f"orow{R}_{t}", tag="orow")
        nc.vector.tensor_add(orow, cr, tmp2)
        if STAGE < 9:
            continue
        nc.sync.dma_start(
            out_d.rearrange("j (n u) -> n j u", u=U)[t * 128:(t + 1) * 128],
            orow.rearrange("p (j u) -> p j u", j=J),
        )


def _prep_shared(weights_output, biases_output, weights_update, biases_update):
    bf = ml_dtypes.bfloat16
    maps = {}
    for tag, W, ob in (("o", weights_output, O1), ("u", weights_update, O2)):
        Wr = W.reshape(66, 5, ob)
        H = Wr[2:, :, :]
        A = Wr[:2, :, :]
        for i in range(5):
            blk = np.concatenate([H[:, i], H[:, i]])   # rows duplicated at 0/64
            maps[f"w{tag}_g{i}"] = np.ascontiguousarray(blk).astype(bf)
        wa = A.transpose(1, 0, 2).reshape(10, ob)
        wa_pad = np.zeros((128, ob), np.float32)
        wa_pad[0:10] = wa
        wa_pad[64:74] = wa
        maps[f"wa_{tag}"] = wa_pad.astype(bf)
    maps["b_o"] = np.ascontiguousarray(biases_output.astype(np.float32)[:, None])
    maps["b_u"] = np.ascontiguousarray(biases_update.astype(np.float32)[:, None])
    return maps


def make_in_maps(inputs, hx, support0, support1, weights_output, biases_output,
                 weights_update, biases_update):
    bf = ml_dtypes.bfloat16
    shared = _prep_shared(np.asarray(weights_output, dtype=np.float32),
                          np.asarray(biases_output, dtype=np.float32),
                          np.asarray(weights_update, dtype=np.float32),
                          np.asarray(biases_update, dtype=np.float32))
    shared["s0t"] = np.ascontiguousarray(np.asarray(support0, np.float32).T).astype(bf)
    shared["s1t"] = np.ascontiguousarray(np.asarray(support1, np.float32).T).astype(bf)

    hx = np.asarray(hx, dtype=np.float32)
    xi = np.asarray(inputs, dtype=np.float32).reshape(B, N, D_IN)
    hx3 = hx.reshape(B, N, U)

    in_maps = []
    for c in range(NCORES):
        sl = slice(c * J, (c + 1) * J)
        hxc = hx3[sl].transpose(1, 0, 2).reshape(N, J * U)
        a0 = xi[sl].transpose(1, 0, 2).reshape(N, 16)   # [n, (j,f)]
        m = dict(shared)
        m["hxr"] = hxc.astype(bf)
        m["hxrf"] = np.ascontiguousarray(hxc)
        m["a0r"] = a0.astype(bf)
        m["a0t"] = np.ascontiguousarray(a0.T).astype(bf)
        in_maps.append(m)
    return in_maps


def kernel(inputs, hx, support0, support1, weights_output, biases_output,
           weights_update, biases_update):
    if "nc" not in _CACHE:
        _CACHE["nc"] = _build()
    nc = _CACHE["nc"]
    in_maps = make_in_maps(inputs, hx, support0, support1, weights_output,
                           biases_output, weights_update, biases_update)
    res = run_bass_kernel_spmd(nc, in_maps, core_ids=list(range(NCORES)))
    return np.concatenate([r["out"] for r in res.results], axis=0)

